# revision 32
# baseline (speedup 1.0000x reference)
"""Trainium2 Bass kernel for nn_Attention_9431748182241.

Module: x -> 1x1 qkv conv -> {3x3,5x5,7x7} depthwise convs -> q/k/v 1x1
projections -> per-head channel attention (CxC over L2-normalized q,k)
-> 1x1 out projection.

Algorithm: the entire pre-attention pipeline is linear in x and collapses
(host-side weight folding) to

    q = sum_{t in 7x7 offsets} Bq_t @ S_t(x)        (same for k, v)

where S_t is the zero-padded spatial shift. On-device, per 4-image-row
spatial tile (512 cols): 3x49 fp32r matmuls accumulate q/k/v in PSUM from
a zero-padded SBUF copy of x read at 49 shifted access patterns. Norms
(ACT square+accum) and the per-head Gram matrix (PE transpose + matmul,
PSUM-accumulated across all tiles) are computed inline; v streams to DRAM.
A tiny finale builds softmax attention per head, folds it with W_out into
a single [128,128] matrix, and a second pass produces out = (W_out A) @ v.

Sharding: data-parallel — batch 8 across 8 cores, identical program (SPMD),
no collectives.
"""

from contextlib import ExitStack

import numpy as np

import concourse.bass as bass
import concourse.bacc as bacc
import concourse.mybir as mybir
import concourse.tile as tile
from concourse.bass_utils import run_bass_kernel_spmd

B, C, H, W = 8, 128, 128, 128
HEADS = 8
DH = C // HEADS  # 16
PAD = 3
NOFF = 49  # 7x7 offset union
TILE_ROWS = 4
GRAM_MODE = "pe_bf16"  # dma_bf16 | pe_bf16
f32 = mybir.dt.float32
f32r = mybir.dt.float32r
bf16 = mybir.dt.bfloat16

_NC_CACHE = {}


def fold_weights(w_qkv, w_dw3, w_dw5, w_dw7, w_q, w_k, w_v):
    """[3, 49, C, C] f64: out_o = sum_t B[o,t] @ S_t(x)."""
    w_qkv = np.asarray(w_qkv, np.float64)
    dws = [np.asarray(w, np.float64) for w in (w_dw3, w_dw5, w_dw7)]
    w_o = [np.asarray(w, np.float64) for w in (w_q, w_k, w_v)]

    Bm = np.zeros((3, NOFF, C, C))
    offsets = [(dy, dx) for dy in range(-3, 4) for dx in range(-3, 4)]
    for o in range(3):
        part = o * C
        V = w_qkv[part : part + C, :]
        for ti, (dy, dx) in enumerate(offsets):
            A = np.zeros((C, C))
            for g, k in enumerate((3, 5, 7)):
                p = k // 2
                if abs(dy) <= p and abs(dx) <= p:
                    taps = dws[g][part : part + C, 0, dy + p, dx + p]
                    A += w_o[o][:, g * C : (g + 1) * C] * taps[None, :]
            Bm[o, ti] = A @ V
    return Bm


def build_nc(h=H, w=W, dbg=False):
    """Build the per-core Bass program. h, w: image dims (w must be 128)."""
    assert w == 128 and h % TILE_ROWS == 0
    hw = h * w
    nt = h // TILE_ROWS
    N = TILE_ROWS * w  # moving-dim per tile
    hp, wp = h + 2 * PAD, w + 2 * PAD

    nc = bacc.Bacc("TRN2", target_bir_lowering=False, debug=False)
    dbg_d = {}
    if dbg:
        for nm, shp, dt_ in [
            ("dq", [C, hw], bf16), ("dk", [C, hw], bf16), ("dg", [C, C], f32),
            ("dabd", [C, C], f32), ("dmf", [C, C], f32), ("dnq", [C, 1], f32),
        ]:
            dbg_d[nm] = nc.dram_tensor(nm, shp, dt_, kind="ExternalOutput")
    x_d = nc.dram_tensor("x", [C, hp * wp], f32, kind="ExternalInput")
    wB_d = nc.dram_tensor("wB", [C, 3 * NOFF * C], f32, kind="ExternalInput")
    woutT_d = nc.dram_tensor("woutT", [C, C], f32, kind="ExternalInput")
    tempc_d = nc.dram_tensor("tempc", [C, 1], f32, kind="ExternalInput")
    ident_d = nc.dram_tensor("ident", [C, C], f32, kind="ExternalInput")
    maskn_d = nc.dram_tensor("maskn", [C, C], f32, kind="ExternalInput")
    zcon_d = nc.dram_tensor("zcon", [C, C], f32, kind="ExternalInput")  # ones
    y_d = nc.dram_tensor("y", [C, hw], f32, kind="ExternalOutput")

    with tile.TileContext(nc) as tc, ExitStack() as ctx:
        sb_x = ctx.enter_context(tc.tile_pool(name="sb_x", bufs=1))
        sb_w = ctx.enter_context(tc.tile_pool(name="sb_w", bufs=1))
        sb_c = ctx.enter_context(tc.tile_pool(name="sb_c", bufs=1))
        sb_qk = ctx.enter_context(tc.tile_pool(name="sb_qk", bufs=4))
        sb_qkT = ctx.enter_context(tc.tile_pool(name="sb_qkT", bufs=4))
        sb_sq = ctx.enter_context(tc.tile_pool(name="sb_sq", bufs=2))
        sb_n = ctx.enter_context(tc.tile_pool(name="sb_n", bufs=1))
        sb_f = ctx.enter_context(tc.tile_pool(name="sb_f", bufs=1))
        sb_v2 = ctx.enter_context(tc.tile_pool(name="sb_v2", bufs=3))
        sb_v3 = ctx.enter_context(tc.tile_pool(name="sb_v3", bufs=16))
        ps_qkv = ctx.enter_context(tc.tile_pool(name="ps_qkv", bufs=5, space="PSUM"))
        ps_tr = ctx.enter_context(tc.tile_pool(name="ps_tr", bufs=2, space="PSUM"))
        ps_g = ctx.enter_context(tc.tile_pool(name="ps_g", bufs=1, space="PSUM"))
        dr_v = ctx.enter_context(tc.tile_pool(name="dr_v", bufs=1, space="DRAM"))

        # ---- constants / inputs into SBUF ----
        zcon = sb_c.tile([C, C], f32, tag="zcon")
        nc.sync.dma_start(zcon[:], zcon_d.ap())
        ones1 = zcon[0:1, 0:C]

        ident = sb_c.tile([C, C], f32, tag="ident")
        nc.sync.dma_start(ident[:], ident_d.ap())
        ident_b = sb_c.tile([C, C], bf16, tag="ident_b")
        nc.vector.tensor_copy(ident_b[:], ident[:])

        # x arrives pre-padded from the host; chunked so early tiles'
        # matmuls start ASAP
        xp = sb_x.tile([C, hp * wp], f32)
        xp3 = xp[:].rearrange("p (a b) -> p a b", b=wp)
        nxc = 8
        bnd = [hp * wp * c // nxc for c in range(nxc + 1)]
        for c0 in range(nxc):
            nc.sync.dma_start(
                xp[:, bnd[c0] : bnd[c0 + 1]].bitcast(f32r),
                x_d.ap()[:, bnd[c0] : bnd[c0 + 1]].bitcast(f32r),
            )
        # weights on the other queue, q block first
        wB = sb_w.tile([C, 3 * NOFF * C], f32)
        for o in range(3):
            nc.scalar.dma_start(
                wB[:, bass.ts(o, NOFF * C)].bitcast(f32r),
                wB_d.ap()[:, bass.ts(o, NOFF * C)].bitcast(f32r),
            )
        woutT = sb_c.tile([C, C], f32, tag="woutT")
        nc.sync.dma_start(woutT[:], woutT_d.ap())
        tempc = sb_c.tile([C, 1], f32, tag="tempc")
        nc.sync.dma_start(tempc[:], tempc_d.ap())
        maskn = sb_c.tile([C, C], f32, tag="maskn")
        nc.sync.dma_start(maskn[:], maskn_d.ap())

        nq_p = sb_n.tile([C, nt], f32, tag="nq_p")
        nk_p = sb_n.tile([C, nt], f32, tag="nk_p")
        v_dram = dr_v.tile([C, hw], f32)

        g_ps = ps_g.tile([C, C], f32)

        offsets = [(dy, dx) for dy in range(-3, 4) for dx in range(-3, 4)]

        def qkv_mms(o, out_ps, i):
            """49 accumulating fp32r matmuls for output o at spatial tile i."""
            y0 = i * TILE_ROWS
            for ti, (dy, dx) in enumerate(offsets):
                rhs = xp3[
                    :,
                    y0 + PAD + dy : y0 + PAD + dy + TILE_ROWS,
                    PAD + dx : PAD + dx + w,
                ]
                nc.tensor.matmul(
                    out_ps[:],
                    wB[:, bass.ts(o * NOFF + ti, C)].bitcast(f32r),
                    rhs.bitcast(f32r),
                    start=(ti == 0),
                    stop=(ti == NOFF - 1),
                )

        # ---- phase 1: q/k/v tiles, norms, Gram ----
        for i in range(nt):
            q_ps = ps_qkv.tile([C, N], f32, tag="qkv")
            qkv_mms(0, q_ps, i)
            k_ps = ps_qkv.tile([C, N], f32, tag="qkv")
            qkv_mms(1, k_ps, i)
            v_ps = ps_qkv.tile([C, N], f32, tag="qkv")
            qkv_mms(2, v_ps, i)

            # v -> SBUF -> DRAM
            v_s = sb_v2.tile([C, N], f32, tag="v2")
            nc.vector.tensor_copy(v_s[:], v_ps[:])
            nc.sync.dma_start(v_dram[:, bass.ts(i, N)], v_s[:])

            # norms: ACT square with accumulate, straight off PSUM
            sq_q = sb_sq.tile([C, N], f32, tag="sq")
            nc.scalar.activation(
                sq_q[:], q_ps[:], mybir.ActivationFunctionType.Square,
                accum_out=nq_p[:, i : i + 1],
            )
            sq_k = sb_sq.tile([C, N], f32, tag="sq")
            nc.scalar.activation(
                sq_k[:], k_ps[:], mybir.ActivationFunctionType.Square,
                accum_out=nk_p[:, i : i + 1],
            )

            # PSUM -> SBUF cast-copies (bf16) for the Gram path; bf16 noise
            # averages out over the 16384-long dot products.
            q_s = sb_qk.tile([C, N], bf16, tag="qk")
            nc.vector.tensor_copy(q_s[:], q_ps[:])
            k_s = sb_qk.tile([C, N], bf16, tag="qk")
            nc.vector.tensor_copy(k_s[:], k_ps[:])

            if dbg:
                nc.sync.dma_start(dbg_d["dq"].ap()[:, bass.ts(i, N)], q_s[:])
                nc.sync.dma_start(dbg_d["dk"].ap()[:, bass.ts(i, N)], k_s[:])

            # transpose 128-chunks; Gram accumulates G += q_chunk @ k_chunk^T
            qT = sb_qkT.tile([C, N], bf16, tag="qkT")
            kT = sb_qkT.tile([C, N], bf16, tag="qkT")
            if GRAM_MODE == "dma_bf16":
                for j in range(N // C):
                    nc.scalar.dma_start_transpose(
                        qT[:, bass.ts(j, C)], q_s[:, bass.ts(j, C)]
                    )
                    nc.scalar.dma_start_transpose(
                        kT[:, bass.ts(j, C)], k_s[:, bass.ts(j, C)]
                    )
            else:
                for j in range(N // C):
                    t_ps = ps_tr.tile([C, C], bf16, tag="tr")
                    nc.tensor.transpose(
                        t_ps[:], q_s[:, bass.ts(j, C)], ident_b[:]
                    )
                    nc.vector.tensor_copy(qT[:, bass.ts(j, C)], t_ps[:])
                    t_ps2 = ps_tr.tile([C, C], bf16, tag="tr")
                    nc.tensor.transpose(
                        t_ps2[:], k_s[:, bass.ts(j, C)], ident_b[:]
                    )
                    nc.vector.tensor_copy(kT[:, bass.ts(j, C)], t_ps2[:])
            for j in range(N // C):
                nc.tensor.matmul(
                    g_ps[:],
                    qT[:, bass.ts(j, C)],
                    kT[:, bass.ts(j, C)],
                    start=(i == 0 and j == 0),
                    stop=(i == nt - 1 and j == N // C - 1),
                )

        # ---- finale: softmax attention + fold with W_out ----
        nq = sb_f.tile([C, 1], f32, tag="nq")
        nc.vector.reduce_sum(nq[:], nq_p[:], axis=mybir.AxisListType.X)
        nk = sb_f.tile([C, 1], f32, tag="nk")
        nc.vector.reduce_sum(nk[:], nk_p[:], axis=mybir.AxisListType.X)
        # 1/||q|| = reciprocal(sqrt(sum q^2)); norms >> eps=1e-12 here
        nq_s = sb_f.tile([C, 1], f32, tag="nq_s")
        nc.scalar.sqrt(nq_s[:], nq[:])
        rq = sb_f.tile([C, 1], f32, tag="rq")
        nc.vector.reciprocal(rq[:], nq_s[:])
        nk_s = sb_f.tile([C, 1], f32, tag="nk_s")
        nc.scalar.sqrt(nk_s[:], nk[:])
        rk = sb_f.tile([C, 1], f32, tag="rk")
        nc.vector.reciprocal(rk[:], nk_s[:])
        # rq2 = rq * temperature(per-channel)
        rq2 = sb_f.tile([C, 1], f32, tag="rq2")
        nc.vector.tensor_mul(rq2[:], rq[:], tempc[:])

        # rk as a row, broadcast down partitions via outer product with ones
        rk_row_ps = ps_tr.tile([C, C], f32, tag="tr")
        nc.tensor.transpose(rk_row_ps[0:1, :], rk[:], ident[:])
        rk_row = sb_f.tile([1, C], f32, tag="rk_row")
        nc.vector.tensor_copy(rk_row[:], rk_row_ps[0:1, :])
        rkb_ps = ps_tr.tile([C, C], f32, tag="tr")
        nc.tensor.matmul(rkb_ps[:], ones1[:], rk_row[:], start=True, stop=True)
        rkb = sb_f.tile([C, C], f32, tag="rkb")
        nc.vector.tensor_copy(rkb[:], rkb_ps[:])

        # masked softmax over the full [C, C] Gram: off-head-block entries
        # get a -1e4 bias -> exp underflows to exactly 0, so the softmax
        # result IS the block-diagonal attention matrix A.
        g_s = sb_f.tile([C, C], f32, tag="g_s")
        nc.vector.tensor_copy(g_s[:], g_ps[:])
        g1 = sb_f.tile([C, C], f32, tag="g1")
        nc.vector.tensor_mul(g1[:], g_s[:], rkb[:])
        g2 = sb_f.tile([C, C], f32, tag="g2")
        nc.vector.tensor_scalar_mul(g2[:], g1[:], rq2[:])
        g3 = sb_f.tile([C, C], f32, tag="g3")
        nc.vector.tensor_add(g3[:], g2[:], maskn[:])
        mx = sb_f.tile([C, 1], f32, tag="mx")
        nc.vector.reduce_max(mx[:], g3[:], axis=mybir.AxisListType.X)
        nmx = sb_f.tile([C, 1], f32, tag="nmx")
        nc.vector.tensor_scalar_mul(nmx[:], mx[:], -1.0)
        ex = sb_f.tile([C, C], f32, tag="ex")
        ssum = sb_f.tile([C, 1], f32, tag="ssum")
        nc.scalar.activation(
            ex[:], g3[:], mybir.ActivationFunctionType.Exp,
            bias=nmx[:], accum_out=ssum[:],
        )
        rs = sb_f.tile([C, 1], f32, tag="rs")
        nc.vector.reciprocal(rs[:], ssum[:])
        a_bd = sb_f.tile([C, C], f32, tag="a_bd")
        nc.vector.tensor_scalar_mul(a_bd[:], ex[:], rs[:])

        if dbg:
            nc.sync.dma_start(dbg_d["dg"].ap(), g_s[:])
            nc.sync.dma_start(dbg_d["dabd"].ap(), a_bd[:])
            nc.sync.dma_start(dbg_d["dnq"].ap(), nq[:])

        # M_final = (W_out A)^T = A^T W_out^T
        mf_ps = ps_tr.tile([C, C], f32, tag="tr")
        nc.tensor.matmul(mf_ps[:], a_bd[:], woutT[:], start=True, stop=True)
        m_final = sb_f.tile([C, C], f32, tag="m_final")
        nc.vector.tensor_copy(m_final[:].bitcast(f32r), mf_ps[:])
        if dbg:
            nc.sync.dma_start(dbg_d["dmf"].ap(), m_final[:])

        # ---- phase 2: out = M_final^T @ v ----
        for i in range(nt):
            v_s = sb_v3.tile([C, N], f32, tag="v3")
            nc.scalar.dma_start(
                v_s[:].bitcast(f32r), v_dram[:, bass.ts(i, N)].bitcast(f32r)
            )
            o_ps = ps_qkv.tile([C, N], f32, tag="qkv")
            nc.tensor.matmul(
                o_ps[:], m_final[:].bitcast(f32r), v_s[:].bitcast(f32r),
                start=True, stop=True,
            )
            o_s = sb_v2.tile([C, N], f32, tag="v2")
            nc.vector.tensor_copy(o_s[:], o_ps[:])
            nc.sync.dma_start(y_d.ap()[:, bass.ts(i, N)], o_s[:])

    nc.compile()
    return nc


def _prep_inputs(inputs, h=H, w=W):
    Bm = fold_weights(
        inputs["w_qkv"], inputs["w_dw3"], inputs["w_dw5"], inputs["w_dw7"],
        inputs["w_q"], inputs["w_k"], inputs["w_v"],
    )
    # lhsT layout: block (o, t) = B[o,t]^T  ([K=in_ch, M=out_ch])
    wB = np.ascontiguousarray(
        Bm.transpose(3, 0, 1, 2).reshape(C, 3 * NOFF * C)
    ).astype(np.float32)  # [C, 3*49*C]
    woutT = np.ascontiguousarray(np.asarray(inputs["w_out"]).T).astype(np.float32)
    tempc = np.repeat(
        np.asarray(inputs["temperature"], np.float32).reshape(HEADS), DH
    ).reshape(C, 1)
    ident = np.eye(C, dtype=np.float32)
    maskn = np.full((C, C), -1e4, np.float32)
    for hd in range(HEADS):
        maskn[hd * DH : (hd + 1) * DH, hd * DH : (hd + 1) * DH] = 0.0
    zcon = np.ones((C, C), np.float32)
    x = np.asarray(inputs["x"], np.float32)
    nb = x.shape[0]
    hp, wp = h + 2 * PAD, w + 2 * PAD
    xpad = np.zeros((nb, C, hp, wp), np.float32)
    xpad[:, :, PAD : PAD + h, PAD : PAD + w] = x.reshape(nb, C, h, w)
    in_maps = [
        {
            "x": np.ascontiguousarray(xpad[b].reshape(C, hp * wp)),
            "wB": wB,
            "woutT": woutT,
            "tempc": tempc,
            "ident": ident,
            "maskn": maskn,
            "zcon": zcon,
        }
        for b in range(nb)
    ]
    return in_maps


def kernel(**inputs):
    if "nc" not in _NC_CACHE:
        _NC_CACHE["nc"] = build_nc()
    nc = _NC_CACHE["nc"]
    in_maps = _prep_inputs(inputs)
    res = run_bass_kernel_spmd(nc, in_maps, core_ids=list(range(B)))
    out = np.stack([res.results[b]["y"].reshape(C, H, W) for b in range(B)])
    return out.astype(np.float32)
